# revision 9
# baseline (speedup 1.0000x reference)
"""Trainium2 Bass kernel for nn_FractalSemanticNetwork.

Self-contained: takes FULL inputs, shards batch over 8 NeuronCores (data
parallel), runs a hand-written Bass/Tile kernel per core, gathers outputs.

Layout strategy per core (B_loc=4 seqs of S=512 -> 2048 tokens, 16 tiles):
  - matmuls "choice B": psum[out_tile, tok] = W^T[k,o].T-contract  (feature-major out)
  - matmuls "choice A": psum[tok_tile, out] with lhsT = activation feature-major
  - residual stream x token-major fp32; LN via bn_stats on DVE
  - attention: S^T computed directly (no P transpose); softmax sums via
    ones-column appended to V; exp on ACT with fused 1/sqrt(d) scale
  - scale-attention (5 "tokens"): elementwise on DVE/GPSIMD (per-position dots)
  - weights bf16, SBUF resident (phase-A set + per-layer rotating)
"""

import os
import sys
import numpy as np

for _p in ("/opt/trn_rl_repo",):
    if _p not in sys.path and os.path.isdir(_p):
        sys.path.insert(0, _p)

import ml_dtypes  # noqa: E402
import concourse.bass as bass  # noqa: E402
import concourse.bacc as bacc  # noqa: E402
import concourse.tile as tile  # noqa: E402
from concourse import mybir  # noqa: E402
from concourse.bass_utils import run_bass_kernel_spmd  # noqa: E402
from concourse.masks import make_identity  # noqa: E402

F32 = mybir.dt.float32
BF16 = mybir.dt.bfloat16
AF = mybir.ActivationFunctionType
OP = mybir.AluOpType
AX = mybir.AxisListType
BFNP = ml_dtypes.bfloat16

E = 256        # embed dim
I_IN = 306     # input feature dim
IP = 384       # padded input feature dim (3 k-tiles)
NT = 128       # n tickers
NS = 5         # n scales
NL = 4         # layers
DFF = 1024
EPS = 1e-5
B, S = 32, 512
NCORES = 8
BL = B // NCORES          # 4 seqs per core
T = BL * S                # 2048 tokens per core
NTILES = T // 128         # 16


def _bf(a):
    return np.ascontiguousarray(np.asarray(a, np.float32)).astype(BFNP)


def _wt(w, pad_in=None):
    """torch W [out,in] -> sbuf layout [128, ktiles, out] (W^T, in on partitions)."""
    w = np.asarray(w, np.float32)
    o, i = w.shape
    wt = w.T  # [in, out]
    if pad_in is not None and pad_in != i:
        p = np.zeros((pad_in, o), np.float32)
        p[:i] = wt
        wt = p
        i = pad_in
    assert i % 128 == 0, i
    return _bf(wt.reshape(i // 128, 128, o).transpose(1, 0, 2))


def _wt_part(w):
    """W [out,in] with in<=128 -> sbuf [in, 1, out]."""
    w = np.asarray(w, np.float32)
    return _bf(w.T[:, None, :])


def _perp(b, pad_to=None):
    """bias [n] -> per-partition layout [128, ntiles] f32 (column per o-tile)."""
    b = np.asarray(b, np.float32)
    n = b.shape[0]
    if pad_to is not None and pad_to != n:
        bb = np.zeros(pad_to, np.float32)
        bb[:n] = b
        b = bb
        n = pad_to
    if n < 128:
        bb = np.zeros(128, np.float32)
        bb[:n] = b
        return np.ascontiguousarray(bb[:, None])
    assert n % 128 == 0
    return np.ascontiguousarray(b.reshape(n // 128, 128).T)


def _repl(v, dtype=np.float32):
    v = np.asarray(v, np.float32)
    out = np.tile(v[None, :], (128, 1))
    return out.astype(BFNP) if dtype is BFNP else np.ascontiguousarray(out)


def _is0(a):
    return bool(np.all(np.asarray(a) == 0.0))


def _is1(a):
    return bool(np.all(np.asarray(a) == 1.0))


def _prep(params):
    """Flatten nested params into named numpy arrays in on-chip layouts."""
    pp = {}
    meta = {}
    sc = params["scales"]
    for s in range(NS):
        pp[f"s{s}_w1t"] = _wt(sc[s]["W1"], pad_in=IP)       # [128,3,256]
        pp[f"s{s}_b1"] = _perp(sc[s]["b1"])                  # [128,2]
        pp[f"s{s}_w2t"] = _wt(sc[s]["W2"])                   # [128,2,256]
        meta[f"s{s}_b2_zero"] = _is0(sc[s]["b2"])
        if not meta[f"s{s}_b2_zero"]:
            pp[f"s{s}_b2r"] = _repl(sc[s]["b2"])
        meta[f"s{s}_ln_id"] = _is1(sc[s]["ln_g"]) and _is0(sc[s]["ln_b"])
        if not meta[f"s{s}_ln_id"]:
            pp[f"s{s}_lngr"] = _repl(sc[s]["ln_g"])
            pp[f"s{s}_lnbr"] = _repl(sc[s]["ln_b"])
    at = params["attn"]
    pp["sa_wqkvt"] = _wt(at["Wqkv"])                          # [128,2,768]
    meta["sa_bqkv_zero"] = _is0(at["bqkv"])
    if not meta["sa_bqkv_zero"]:
        pp["sa_bqkvr"] = _repl(at["bqkv"])
    pp["sa_wot"] = _wt(at["Wo"])                              # [128,2,256]
    meta["sa_bo_zero"] = _is0(at["bo"])
    if not meta["sa_bo_zero"]:
        pp["sa_bor"] = _repl(at["bo"])
    pp["g_wt"] = _wt(params["gate"]["W"])                     # [128,10,256]
    meta["g_b_zero"] = _is0(params["gate"]["b"])
    if not meta["g_b_zero"]:
        pp["g_br"] = _repl(params["gate"]["b"])
    pp["p_wt"] = _wt(params["proj"]["W"])                     # [128,10,256]
    meta["p_b_zero"] = _is0(params["proj"]["b"])
    if not meta["p_b_zero"]:
        pp["p_br"] = _repl(params["proj"]["b"])
    meta["p_ln_id"] = _is1(params["proj"]["ln_g"]) and _is0(params["proj"]["ln_b"])
    if not meta["p_ln_id"]:
        pp["p_lngr"] = _repl(params["proj"]["ln_g"])
        pp["p_lnbr"] = _repl(params["proj"]["ln_b"])
    for l in range(NL):
        L = params["layers"][l]
        pp[f"l{l}_wqkvt"] = _wt(L["Wqkv"])                    # [128,2,768]
        pp[f"l{l}_bqkv"] = _perp(L["bqkv"])                   # [128,6] (q,k per-part; v cols 4:6 unused)
        meta[f"l{l}_bv_zero"] = _is0(L["bqkv"][2 * E:])
        if not meta[f"l{l}_bv_zero"]:
            pp[f"l{l}_bvr"] = _repl(L["bqkv"][2 * E:])
        pp[f"l{l}_wot"] = _wt(L["Wo"])                        # [128,2,256]
        meta[f"l{l}_bo_zero"] = _is0(L["bo"])
        if not meta[f"l{l}_bo_zero"]:
            pp[f"l{l}_bor"] = _repl(L["bo"])
        pp[f"l{l}_wf1t"] = _wt(L["Wf1"])                      # [128,2,1024]
        pp[f"l{l}_bf1"] = _perp(L["bf1"])                     # [128,8]
        pp[f"l{l}_wf2t"] = _wt(L["Wf2"])                      # [128,8,256]
        meta[f"l{l}_bf2_zero"] = _is0(L["bf2"])
        if not meta[f"l{l}_bf2_zero"]:
            pp[f"l{l}_bf2r"] = _repl(L["bf2"])
        meta[f"l{l}_ln1_id"] = _is1(L["ln1_g"]) and _is0(L["ln1_b"])
        if not meta[f"l{l}_ln1_id"]:
            pp[f"l{l}_ln1gr"] = _repl(L["ln1_g"])
            pp[f"l{l}_ln1br"] = _repl(L["ln1_b"])
        meta[f"l{l}_ln2_id"] = _is1(L["ln2_g"]) and _is0(L["ln2_b"])
        if not meta[f"l{l}_ln2_id"]:
            pp[f"l{l}_ln2gr"] = _repl(L["ln2_g"])
            pp[f"l{l}_ln2br"] = _repl(L["ln2_b"])
    pp["ret_w1t"] = _wt(params["ret"]["W1"])                  # [128,2,256]
    pp["ret_b1"] = _perp(params["ret"]["b1"])
    pp["ret_w2t"] = _wt(params["ret"]["W2"])                  # [128,2,128]
    pp["ret_b2"] = _perp(params["ret"]["b2"])
    pp["reg_w1t"] = _wt(params["reg"]["W1"])                  # [128,2,128]
    pp["reg_b1"] = _perp(params["reg"]["b1"])
    pp["reg_w2t"] = _wt_part(params["reg"]["W2"])             # [128,1,3]
    pp["reg_b2"] = _perp(params["reg"]["b2"])
    pp["hur_w1t"] = _wt(params["hur"]["W1"])                  # [128,2,64]
    pp["hur_b1"] = _perp(params["hur"]["b1"])
    pp["hur_w2t"] = _wt_part(params["hur"]["W2"])             # [64,1,128]
    pp["hur_b2"] = _perp(params["hur"]["b2"])
    return pp, meta


def _ln_tm(nc, pools, xr, out, eps_tile, g_repl=None, b_repl=None):
    """Token-major LN over last dim (free). xr: [128, E] f32 sbuf. out: [128,E]."""
    st = pools["ln"].tile([128, 6], F32, tag="ln_st", name="ln_st")
    mv = pools["ln"].tile([128, 2], F32, tag="ln_mv", name="ln_mv")
    nc.vector.bn_stats(out=st, in_=xr)
    nc.vector.bn_aggr(out=mv, in_=st)
    sd = pools["ln"].tile([128, 1], F32, tag="ln_sd", name="ln_sd")
    nc.scalar.activation(out=sd, in_=mv[:, 1:2], func=AF.Sqrt, bias=eps_tile)
    rstd = pools["ln"].tile([128, 1], F32, tag="ln_rs", name="ln_rs")
    nc.vector.reciprocal(out=rstd, in_=sd)
    if g_repl is None:
        nc.vector.tensor_scalar(out=out, in0=xr, scalar1=mv[:, 0:1], scalar2=rstd,
                                op0=OP.subtract, op1=OP.mult)
    else:
        u = pools["ln"].tile([128, E], F32, tag="ln_u", name="ln_u")
        nc.vector.tensor_scalar(out=u, in0=xr, scalar1=mv[:, 0:1], scalar2=rstd,
                                op0=OP.subtract, op1=OP.mult)
        nc.vector.tensor_mul(out=u, in0=u, in1=g_repl)
        nc.vector.tensor_add(out=out, in0=u, in1=b_repl)


def _build(pp, meta, dbg=False):
    nc = bacc.Bacc("TRN2", target_bir_lowering=False)
    feat_in = nc.declare_dram_parameter("feat", [IP, T], BF16, isOutput=False)
    o_ret = nc.declare_dram_parameter("out_ret", [BL, NT], F32, isOutput=True)
    o_reg = nc.declare_dram_parameter("out_reg", [BL, 3], F32, isOutput=True)
    o_hur = nc.declare_dram_parameter("out_hur", [BL, NT], F32, isOutput=True)
    o_fin = nc.declare_dram_parameter("out_final", [BL, E], F32, isOutput=True)
    dbg_t = {}
    if dbg:
        dbg_t["xA"] = nc.dram_tensor("dbg_xA", [128, NTILES, E], F32, kind="ExternalOutput")
        for l in range(NL):
            dbg_t[f"x{l}"] = nc.dram_tensor(f"dbg_x{l}", [128, NTILES, E], F32, kind="ExternalOutput")

    dr = {k: nc.inline_tensor(np.ascontiguousarray(v), name=k) for k, v in pp.items()}

    with tile.TileContext(nc) as tc:
        from contextlib import ExitStack
        with ExitStack() as ctx:
            com = ctx.enter_context(tc.tile_pool(name="com", bufs=1))
            psA = ctx.enter_context(tc.tile_pool(name="psA", bufs=2, space="PSUM"))
            psB = ctx.enter_context(tc.tile_pool(name="psB", bufs=2, space="PSUM"))
            psT = ctx.enter_context(tc.tile_pool(name="psT", bufs=2, space="PSUM"))
            psS = ctx.enter_context(tc.tile_pool(name="psS", bufs=2, space="PSUM"))
            lnp = ctx.enter_context(tc.tile_pool(name="lnp", bufs=4))
            pools = {"ln": lnp}

            ident = com.tile([128, 128], BF16, name="ident")
            make_identity(nc, ident)
            eps_tile = com.tile([128, 1], F32, name="eps_tile")
            nc.vector.memset(eps_tile, EPS)

            def load(name, shape, dtype=BF16, pool=None):
                t = (pool or com).tile(list(shape), dtype, name=name, tag=name)
                nc.sync.dma_start(out=t, in_=dr[name][:])
                return t

            # ---- resident small/common weights
            W = {}
            for nm, shp, dt_ in [
                ("ret_w1t", (128, 2, E), BF16), ("ret_b1", (128, 2), F32),
                ("ret_w2t", (128, 2, NT), BF16), ("ret_b2", (128, 1), F32),
                ("reg_w1t", (128, 2, NT), BF16), ("reg_b1", (128, 1), F32),
                ("reg_w2t", (128, 1, 3), BF16), ("reg_b2", (128, 1), F32),
                ("hur_w1t", (128, 2, 64), BF16), ("hur_b1", (128, 1), F32),
                ("hur_w2t", (64, 1, 128), BF16), ("hur_b2", (128, 1), F32),
            ]:
                W[nm] = load(nm, shp, dt_)

            # residual stream, one tile per sequence to keep chunks independent
            x_c = [com.tile([128, 4, E], F32, name=f"x_c{c}", tag=f"x_c{c}")
                   for c in range(BL)]
            f_tm = com.tile([BL, E], F32, name="f_tm")         # final tokens

            # =====================  PHASE A  =====================
            with ExitStack() as actx:
                pa = actx.enter_context(tc.tile_pool(name="pa", bufs=1))
                pav = actx.enter_context(tc.tile_pool(name="pav", bufs=2))
                PA = {}
                for s in range(NS):
                    PA[f"s{s}_w1t"] = load(f"s{s}_w1t", (128, 3, E), pool=pa)
                    PA[f"s{s}_b1"] = load(f"s{s}_b1", (128, 2), F32, pool=pa)
                    PA[f"s{s}_w2t"] = load(f"s{s}_w2t", (128, 2, E), pool=pa)
                    if not meta[f"s{s}_b2_zero"]:
                        PA[f"s{s}_b2r"] = load(f"s{s}_b2r", (128, E), F32, pool=pa)
                    if not meta[f"s{s}_ln_id"]:
                        PA[f"s{s}_lngr"] = load(f"s{s}_lngr", (128, E), F32, pool=pa)
                        PA[f"s{s}_lnbr"] = load(f"s{s}_lnbr", (128, E), F32, pool=pa)
                PA["sa_wqkvt"] = load("sa_wqkvt", (128, 2, 3 * E), pool=pa)
                if not meta["sa_bqkv_zero"]:
                    PA["sa_bqkvr"] = load("sa_bqkvr", (128, 3 * E), F32, pool=pa)
                PA["sa_wot"] = load("sa_wot", (128, 2, E), pool=pa)
                if not meta["sa_bo_zero"]:
                    PA["sa_bor"] = load("sa_bor", (128, E), F32, pool=pa)
                PA["g_wt"] = load("g_wt", (128, 10, E), pool=pa)
                PA["p_wt"] = load("p_wt", (128, 10, E), pool=pa)
                if not meta["g_b_zero"]:
                    PA["g_br"] = load("g_br", (128, E), F32, pool=pa)
                if not meta["p_b_zero"]:
                    PA["p_br"] = load("p_br", (128, E), F32, pool=pa)
                if not meta["p_ln_id"]:
                    PA["p_lngr"] = load("p_lngr", (128, E), F32, pool=pa)
                    PA["p_lnbr"] = load("p_lnbr", (128, E), F32, pool=pa)

                for c in range(BL):
                    fsb = pav.tile([128, 3, 512], BF16, tag="fsb", name="fsb")
                    nc.sync.dma_start(
                        out=fsb,
                        in_=feat_in[:].rearrange("(k p) t -> p k t", p=128)[:, :, c * 512:(c + 1) * 512])
                    z_fm = pav.tile([128, NS, 2, 512], BF16, tag="z_fm", name="z_fm")
                    for s in range(NS):
                        # W1 + gelu (choice B)
                        h_fm = pav.tile([128, 2, 512], BF16, tag="h_fm", name="h_fm")
                        for o in range(2):
                            ps = psA.tile([128, 512], F32, tag="psA", name="ps_w1")
                            for k in range(3):
                                nc.tensor.matmul(ps, lhsT=PA[f"s{s}_w1t"][:, k, o * 128:(o + 1) * 128],
                                                 rhs=fsb[:, k, :], start=(k == 0), stop=(k == 2))
                            nc.scalar.activation(out=h_fm[:, o, :], in_=ps, func=AF.Gelu,
                                                 bias=PA[f"s{s}_b1"][:, o:o + 1])
                        # W2 (choice A) + LN -> z (token-major), then transpose to z_fm
                        for q in range(4):
                            ps = psB.tile([128, E], F32, tag="psB", name="ps_w2")
                            for k in range(2):
                                nc.tensor.matmul(ps, lhsT=h_fm[:, k, q * 128:(q + 1) * 128],
                                                 rhs=PA[f"s{s}_w2t"][:, k, :], start=(k == 0), stop=(k == 1))
                            zb = pav.tile([128, E], F32, tag="zb", name="zb")
                            if meta[f"s{s}_b2_zero"]:
                                nc.scalar.copy(out=zb, in_=ps)
                            else:
                                nc.vector.tensor_add(out=zb, in0=ps, in1=PA[f"s{s}_b2r"])
                            z_tm = pav.tile([128, E], BF16, tag="z_tm", name="z_tm")
                            _ln_tm(nc, pools, zb, z_tm, eps_tile,
                                   None if meta[f"s{s}_ln_id"] else PA[f"s{s}_lngr"],
                                   None if meta[f"s{s}_ln_id"] else PA[f"s{s}_lnbr"])
                            for o in range(2):
                                pt = psT.tile([128, 128], BF16, tag="psT", name="ps_zT")
                                nc.tensor.transpose(pt, z_tm[:, o * 128:(o + 1) * 128], ident)
                                nc.scalar.copy(out=z_fm[:, s, o, q * 128:(q + 1) * 128], in_=pt)

                    flat_fm = pav.tile([128, 10, 512], BF16, tag="flat_fm", name="flat_fm")
                    for q in range(4):
                        # scale-attn qkv (choice A): per scale -> qkv_tm [128, NS, 768]
                        qkv_tm = pav.tile([128, NS, 3 * E], BF16, tag="qkv_tm", name="qkv_tm")
                        for s in range(NS):
                            for nh in range(2):
                                ps = psA.tile([128, 512], F32, tag="psA", name="ps_qkv")
                                for k in range(2):
                                    nc.tensor.matmul(ps[:, 0:384],
                                                     lhsT=z_fm[:, s, k, q * 128:(q + 1) * 128],
                                                     rhs=PA["sa_wqkvt"][:, k, nh * 384:(nh + 1) * 384],
                                                     start=(k == 0), stop=(k == 1))
                                if meta["sa_bqkv_zero"]:
                                    nc.scalar.copy(out=qkv_tm[:, s, nh * 384:(nh + 1) * 384], in_=ps[:, 0:384])
                                else:
                                    nc.vector.tensor_add(out=qkv_tm[:, s, nh * 384:(nh + 1) * 384],
                                                         in0=ps[:, 0:384],
                                                         in1=PA["sa_bqkvr"][:, nh * 384:(nh + 1) * 384])
                        # scores: prod_i = q_i (bcast j) * k_j  -> head-reduce
                        sc = pav.tile([128, NS, NS, 4], F32, tag="sc", name="sc")
                        for i in range(NS):
                            prod = pav.tile([128, NS, E], BF16, tag="prod", name="prod")
                            q_i = qkv_tm[:, i, 0:E].unsqueeze(1).broadcast_to([128, NS, E])
                            k_all = qkv_tm[:, :, E:2 * E]
                            nc.gpsimd.tensor_mul(out=prod, in0=q_i, in1=k_all)
                            nc.vector.tensor_reduce(
                                out=sc[:, i, :, :],
                                in_=prod.rearrange("p j (h c) -> p j h c", h=4),
                                axis=AX.X, op=OP.add)
                        P = pav.tile([128, NS, NS, 4], BF16, tag="Pexp", name="Pexp")
                        nc.scalar.activation(out=P, in_=sc, func=AF.Exp, scale=0.125)
                        rs = pav.tile([128, NS, 4], F32, tag="rs", name="rs")
                        nc.vector.tensor_reduce(out=rs, in_=P.transpose([0, 1, 3, 2]),
                                                axis=AX.X, op=OP.add)
                        rr = pav.tile([128, NS, 4], F32, tag="rr", name="rr")
                        nc.vector.reciprocal(out=rr, in_=rs)
                        # AV: avp_i[h,c,j] = v[j,(h c)] * P[i,j,h]; reduce over j; * rr
                        o_nrm = pav.tile([128, NS, E], BF16, tag="o_nrm", name="o_nrm")
                        v_all = qkv_tm[:, :, 2 * E:3 * E].rearrange("p j (h c) -> p j h c", h=4).transpose([0, 2, 3, 1])
                        for i in range(NS):
                            avp = pav.tile([128, 4, 64, NS], BF16, tag="avp", name="avp")
                            P_i = P[:, i, :, :].transpose([0, 2, 1]).unsqueeze(2).broadcast_to([128, 4, 64, NS])
                            nc.gpsimd.tensor_mul(out=avp, in0=v_all, in1=P_i)
                            ored = pav.tile([128, 4, 64], F32, tag="ored", name="ored")
                            nc.vector.tensor_reduce(out=ored, in_=avp, axis=AX.X, op=OP.add)
                            rr_i = rr[:, i, :].unsqueeze(2).broadcast_to([128, 4, 64])
                            nc.vector.tensor_mul(out=o_nrm[:, i, :].rearrange("p (h c) -> p h c", h=4),
                                                 in0=ored, in1=rr_i)
                        # Wo per scale (choice A) -> flat (attn out), then transpose to flat_fm
                        for s in range(NS):
                            of = pav.tile([128, 2, 128], BF16, tag="of", name="of")
                            for o in range(2):
                                pt = psT.tile([128, 128], BF16, tag="psT", name="ps_oT")
                                nc.tensor.transpose(pt, o_nrm[:, s, o * 128:(o + 1) * 128], ident)
                                nc.scalar.copy(out=of[:, o, :], in_=pt)
                            ps = psB.tile([128, E], F32, tag="psB", name="ps_wo")
                            for k in range(2):
                                nc.tensor.matmul(ps, lhsT=of[:, k, :], rhs=PA["sa_wot"][:, k, :],
                                                 start=(k == 0), stop=(k == 1))
                            wo_tm = pav.tile([128, E], BF16, tag="wo_tm", name="wo_tm")
                            if meta["sa_bo_zero"]:
                                nc.scalar.copy(out=wo_tm, in_=ps)
                            else:
                                nc.vector.tensor_add(out=wo_tm, in0=ps, in1=PA["sa_bor"])
                            for o in range(2):
                                pt = psT.tile([128, 128], BF16, tag="psT", name="ps_fT")
                                nc.tensor.transpose(pt, wo_tm[:, o * 128:(o + 1) * 128], ident)
                                nc.scalar.copy(out=flat_fm[:, s * 2 + o, q * 128:(q + 1) * 128], in_=pt)

                    # gate / proj / LN / mul  (choice A per tile)
                    for q in range(4):
                        psg = psB.tile([128, E], F32, tag="psB", name="ps_g")
                        for kk in range(10):
                            nc.tensor.matmul(psg, lhsT=flat_fm[:, kk, q * 128:(q + 1) * 128],
                                             rhs=PA["g_wt"][:, kk, :], start=(kk == 0), stop=(kk == 9))
                        g_tm = pav.tile([128, E], BF16, tag="g_tm", name="g_tm")
                        if meta["g_b_zero"]:
                            nc.scalar.activation(out=g_tm, in_=psg, func=AF.Sigmoid)
                        else:
                            gb = pav.tile([128, E], F32, tag="gb", name="gb")
                            nc.vector.tensor_add(out=gb, in0=psg, in1=PA["g_br"])
                            nc.scalar.activation(out=g_tm, in_=gb, func=AF.Sigmoid)
                        psp = psB.tile([128, E], F32, tag="psB", name="ps_p")
                        for kk in range(10):
                            nc.tensor.matmul(psp, lhsT=flat_fm[:, kk, q * 128:(q + 1) * 128],
                                             rhs=PA["p_wt"][:, kk, :], start=(kk == 0), stop=(kk == 9))
                        pj = pav.tile([128, E], F32, tag="pj", name="pj")
                        if meta["p_b_zero"]:
                            nc.scalar.copy(out=pj, in_=psp)
                        else:
                            nc.vector.tensor_add(out=pj, in0=psp, in1=PA["p_br"])
                        xln = pav.tile([128, E], F32, tag="xln", name="xln")
                        _ln_tm(nc, pools, pj, xln, eps_tile,
                               None if meta["p_ln_id"] else PA["p_lngr"],
                               None if meta["p_ln_id"] else PA["p_lnbr"])
                        nc.vector.tensor_mul(out=x_c[c][:, q, :], in0=xln, in1=g_tm)

            if dbg:
                for c in range(BL):
                    nc.sync.dma_start(out=dbg_t["xA"][:, c * 4:(c + 1) * 4, :], in_=x_c[c])

            # =====================  PHASE B: transformer layers =====================
            with ExitStack() as bctx:
                lw = bctx.enter_context(tc.tile_pool(name="lw", bufs=2))
                pb = bctx.enter_context(tc.tile_pool(name="pb", bufs=2))
                for l in range(NL):
                    wqkv = lw.tile([128, 2, 3 * E], BF16, tag="wqkv", name=f"wqkv{l}")
                    nc.sync.dma_start(out=wqkv, in_=dr[f"l{l}_wqkvt"][:])
                    bqkv = lw.tile([128, 6], F32, tag="bqkv", name=f"bqkv{l}")
                    nc.sync.dma_start(out=bqkv, in_=dr[f"l{l}_bqkv"][:])
                    wo = lw.tile([128, 2, E], BF16, tag="wo", name=f"wo{l}")
                    nc.sync.dma_start(out=wo, in_=dr[f"l{l}_wot"][:])
                    wf1 = lw.tile([128, 2, DFF], BF16, tag="wf1", name=f"wf1{l}")
                    nc.sync.dma_start(out=wf1, in_=dr[f"l{l}_wf1t"][:])
                    bf1 = lw.tile([128, 8], F32, tag="bf1", name=f"bf1{l}")
                    nc.sync.dma_start(out=bf1, in_=dr[f"l{l}_bf1"][:])
                    wf2 = lw.tile([128, 8, E], BF16, tag="wf2", name=f"wf2{l}")
                    nc.sync.dma_start(out=wf2, in_=dr[f"l{l}_wf2t"][:])
                    xb = {}
                    for nm in ["bvr", "bor", "bf2r", "ln1gr", "ln1br", "ln2gr", "ln2br"]:
                        key = f"l{l}_{nm}"
                        if key in pp:
                            xb[nm] = lw.tile([128, E], F32, tag=nm, name=f"{nm}{l}")
                            nc.sync.dma_start(out=xb[nm], in_=dr[key][:])

                    last = (l == NL - 1)
                    NQ = 128 if last else 512
                    qts = [3] if last else [0, 1, 2, 3]
                    for c in range(BL):
                        base = c * 4
                        # x -> x_fm (bf16) for qkv
                        x_fm = pb.tile([128, 2, 512], BF16, tag="x_fm", name="x_fm")
                        for q in range(4):
                            xbf = pb.tile([128, E], BF16, tag="xbf", name="xbf")
                            nc.vector.tensor_copy(out=xbf, in_=x_c[c][:, q, :])
                            for o in range(2):
                                pt = psT.tile([128, 128], BF16, tag="psT", name="ps_xT")
                                nc.tensor.transpose(pt, xbf[:, o * 128:(o + 1) * 128], ident)
                                nc.vector.tensor_copy(out=x_fm[:, o, q * 128:(q + 1) * 128], in_=pt)
                        # q,k (choice B)
                        q_fm = pb.tile([128, 2, NQ], BF16, tag="q_fm", name="q_fm")
                        k_fm = pb.tile([128, 2, 512], BF16, tag="k_fm", name="k_fm")
                        for o in range(2):
                            ps = psA.tile([128, 512], F32, tag="psA", name="ps_q")
                            for k in range(2):
                                nc.tensor.matmul(ps[:, 0:NQ], lhsT=wqkv[:, k, o * 128:(o + 1) * 128],
                                                 rhs=(x_fm[:, k, 384:512] if last else x_fm[:, k, :]),
                                                 start=(k == 0), stop=(k == 1))
                            nc.scalar.activation(out=q_fm[:, o, :], in_=ps[:, 0:NQ], func=AF.Identity,
                                                 bias=bqkv[:, o:o + 1])
                        for o in range(2):
                            ps = psA.tile([128, 512], F32, tag="psA", name="ps_k")
                            for k in range(2):
                                nc.tensor.matmul(ps, lhsT=wqkv[:, k, 256 + o * 128:256 + (o + 1) * 128],
                                                 rhs=x_fm[:, k, :], start=(k == 0), stop=(k == 1))
                            nc.scalar.activation(out=k_fm[:, o, :], in_=ps, func=AF.Identity,
                                                 bias=bqkv[:, 2 + o:3 + o])
                        # v (choice A) -> v_tm [128, 4, 8, 33] with ones col
                        v_tm = pb.tile([128, 4, 8, 33], BF16, tag="v_tm", name="v_tm")
                        nc.vector.memset(v_tm[:, :, :, 32:33], 1.0)
                        for kt in range(4):
                            ps = psB.tile([128, E], F32, tag="psB", name="ps_v")
                            for k in range(2):
                                nc.tensor.matmul(ps, lhsT=x_fm[:, k, kt * 128:(kt + 1) * 128],
                                                 rhs=wqkv[:, k, 2 * E:3 * E], start=(k == 0), stop=(k == 1))
                            dst = v_tm[:, kt, :, 0:32]
                            if meta[f"l{l}_bv_zero"]:
                                nc.vector.tensor_copy(out=dst, in_=ps.rearrange("p (h c) -> p h c", h=8))
                            else:
                                nc.vector.tensor_add(out=dst, in0=ps.rearrange("p (h c) -> p h c", h=8),
                                                     in1=xb["bvr"].rearrange("p (h c) -> p h c", h=8))
                        # attention per head
                        attn_tm = pb.tile([128, len(qts), E], BF16, tag="attn_tm", name="attn_tm")
                        for h in range(8):
                            ot, row = h // 4, (h % 4) * 32
                            expS = pb.tile([128, 4, NQ], BF16, tag="expS", name="expS")
                            for kt in range(4):
                                ps = psA.tile([128, 512], F32, tag="psA", name="ps_s")
                                nc.tensor.matmul(ps[:, 0:NQ],
                                                 lhsT=k_fm[row:row + 32, ot, kt * 128:(kt + 1) * 128],
                                                 rhs=q_fm[row:row + 32, ot, :],
                                                 start=True, stop=True,
                                                 tile_position=(row, 0))
                                nc.scalar.activation(out=expS[:, kt, :], in_=ps[:, 0:NQ], func=AF.Exp,
                                                     scale=float(1.0 / np.sqrt(32.0)))
                            for mi, mq in enumerate(range(NQ // 128)):
                                psv = psS.tile([128, 64], F32, tag="psS", name="ps_av")
                                for kt in range(4):
                                    nc.tensor.matmul(psv[:, 0:33], lhsT=expS[:, kt, mq * 128:(mq + 1) * 128],
                                                     rhs=v_tm[:, kt, h, :], start=(kt == 0), stop=(kt == 3))
                                rrv = pb.tile([128, 1], F32, tag="rrv", name="rrv")
                                nc.vector.reciprocal(out=rrv, in_=psv[:, 32:33])
                                nc.vector.tensor_scalar_mul(out=attn_tm[:, mi, h * 32:(h + 1) * 32],
                                                            in0=psv[:, 0:32], scalar1=rrv)
                        # Wo + residual + LN1 per query tile
                        for mi, q in enumerate(qts):
                            ao_fm = pb.tile([128, 2, 128], BF16, tag="ao_fm", name="ao_fm")
                            for o in range(2):
                                pt = psT.tile([128, 128], BF16, tag="psT", name="ps_aT")
                                nc.tensor.transpose(pt, attn_tm[:, mi, o * 128:(o + 1) * 128], ident)
                                nc.vector.tensor_copy(out=ao_fm[:, o, :], in_=pt)
                            ps = psB.tile([128, E], F32, tag="psB", name="ps_wo2")
                            for k in range(2):
                                nc.tensor.matmul(ps, lhsT=ao_fm[:, k, :], rhs=wo[:, k, :],
                                                 start=(k == 0), stop=(k == 1))
                            xr = pb.tile([128, E], F32, tag="xr", name="xr")
                            nc.vector.tensor_add(out=xr, in0=ps, in1=x_c[c][:, q, :])
                            if not meta[f"l{l}_bo_zero"]:
                                nc.vector.tensor_add(out=xr, in0=xr, in1=xb["bor"])
                            _ln_tm(nc, pools, xr, x_c[c][:, q, :], eps_tile,
                                   xb.get("ln1gr"), xb.get("ln1br"))
                        # FFN + LN2
                        x2_fm = pb.tile([128, 2, NQ], BF16, tag="x2_fm", name="x2_fm")
                        for mi, q in enumerate(qts):
                            xbf2 = pb.tile([128, E], BF16, tag="xbf2", name="xbf2")
                            nc.vector.tensor_copy(out=xbf2, in_=x_c[c][:, q, :])
                            for o in range(2):
                                pt = psT.tile([128, 128], BF16, tag="psT", name="ps_x2T")
                                nc.tensor.transpose(pt, xbf2[:, o * 128:(o + 1) * 128], ident)
                                nc.vector.tensor_copy(out=x2_fm[:, o, mi * 128:(mi + 1) * 128], in_=pt)
                        h_fm = pb.tile([128, 8, NQ], BF16, tag="hf_fm", name="hf_fm")
                        for o in range(8):
                            ps = psA.tile([128, 512], F32, tag="psA", name="ps_f1")
                            for k in range(2):
                                nc.tensor.matmul(ps[:, 0:NQ], lhsT=wf1[:, k, o * 128:(o + 1) * 128],
                                                 rhs=x2_fm[:, k, :], start=(k == 0), stop=(k == 1))
                            nc.scalar.activation(out=h_fm[:, o, :], in_=ps[:, 0:NQ], func=AF.Gelu,
                                                 bias=bf1[:, o:o + 1])
                        for mi, q in enumerate(qts):
                            ps = psB.tile([128, E], F32, tag="psB", name="ps_f2")
                            for k in range(8):
                                nc.tensor.matmul(ps, lhsT=h_fm[:, k, mi * 128:(mi + 1) * 128],
                                                 rhs=wf2[:, k, :], start=(k == 0), stop=(k == 7))
                            xr2 = pb.tile([128, E], F32, tag="xr2", name="xr2")
                            nc.vector.tensor_add(out=xr2, in0=ps, in1=x_c[c][:, q, :])
                            if not meta[f"l{l}_bf2_zero"]:
                                nc.vector.tensor_add(out=xr2, in0=xr2, in1=xb["bf2r"])
                            _ln_tm(nc, pools, xr2, x_c[c][:, q, :], eps_tile,
                                   xb.get("ln2gr"), xb.get("ln2br"))
                    if dbg:
                        for c in range(BL):
                            nc.sync.dma_start(out=dbg_t[f"x{l}"][:, c * 4:(c + 1) * 4, :], in_=x_c[c])

            # =====================  PHASE C: heads =====================
            for c in range(BL):
                nc.sync.dma_start(out=f_tm[c:c + 1, :], in_=x_c[c][127:128, 3, :])
            nc.sync.dma_start(out=o_fin[:], in_=f_tm)
            f_bf = com.tile([BL, E], BF16, name="f_bf")
            nc.vector.tensor_copy(out=f_bf, in_=f_tm)
            f_fm = com.tile([128, 2, BL], BF16, name="f_fm")
            for o in range(2):
                pt = psT.tile([128, 128], BF16, tag="psT", name="ps_hT")
                nc.tensor.transpose(pt[:, 0:BL], f_bf[0:BL, o * 128:(o + 1) * 128], ident[0:BL, 0:BL])
                nc.vector.tensor_copy(out=f_fm[:, o, :], in_=pt[:, 0:BL])

            def head2(pref, mid_tiles, mid_dim, out_dram, out_n, act_out):
                hh = com.tile([128, mid_tiles, BL], BF16, name=f"{pref}_h")
                mp = min(128, mid_dim)
                for o in range(mid_tiles):
                    ps = psS.tile([128, 64], F32, tag="psS", name=f"ps_{pref}1")
                    for k in range(2):
                        lhsT = (W[f"{pref}_w1t"][:, k, o * 128:(o + 1) * 128]
                                if mid_dim > 128 else W[f"{pref}_w1t"][:, k, 0:mid_dim])
                        nc.tensor.matmul(ps[0:mp, 0:BL], lhsT=lhsT,
                                         rhs=f_fm[:, k, :], start=(k == 0), stop=(k == 1))
                    nc.scalar.activation(out=hh[:mp, o, :], in_=ps[:mp, 0:BL], func=AF.Gelu,
                                         bias=W[f"{pref}_b1"][:mp, o:o + 1])
                # second linear
                kt2 = W[f"{pref}_w2t"].shape[1]
                ps = psS.tile([128, 64], F32, tag="psS", name=f"ps_{pref}2")
                if mid_dim > 128:
                    for k in range(kt2):
                        nc.tensor.matmul(ps[0:out_n, 0:BL], lhsT=W[f"{pref}_w2t"][:, k, :],
                                         rhs=hh[:, k, :], start=(k == 0), stop=(k == kt2 - 1))
                else:
                    nc.tensor.matmul(ps[0:out_n, 0:BL], lhsT=W[f"{pref}_w2t"][0:mid_dim, 0, :],
                                     rhs=hh[0:mid_dim, 0, :], start=True, stop=True)
                ob = com.tile([128, BL], F32, name=f"{pref}_ob")
                nc.scalar.activation(out=ob[0:out_n, :], in_=ps[0:out_n, 0:BL], func=act_out,
                                     bias=W[f"{pref}_b2"][0:out_n, 0:1])
                nc.sync.dma_start(out=out_dram[:].transpose([1, 0]), in_=ob[0:out_n, :])

            head2("ret", 2, E, o_ret, NT, AF.Identity)
            head2("reg", 1, NT, o_reg, 3, AF.Identity)
            head2("hur", 1, 64, o_hur, NT, AF.Sigmoid)
    nc.compile()
    return nc


_CACHE = {}


def _get_nc(params, dbg=False):
    key = ("nc", dbg)
    if key not in _CACHE:
        pp, meta = _prep(params)
        _CACHE[key] = (_build(pp, meta, dbg=dbg), meta)
    return _CACHE[key][0]


def _make_runner(nc):
    """Build a reusable jitted SPMD runner (mirrors bass2jax.run_bass_via_pjrt)."""
    import jax
    from jax.sharding import Mesh, PartitionSpec
    from jax.experimental.shard_map import shard_map
    from concourse import bass2jax

    bass2jax.install_neuronx_cc_hook()
    pname = nc.partition_id_tensor.name if nc.partition_id_tensor else None
    in_names, out_names, out_avals, zero_outs = [], [], [], []
    for alloc in nc.m.functions[0].allocations:
        if not isinstance(alloc, mybir.MemoryLocationSet):
            continue
        name = alloc.memorylocations[0].name
        if alloc.kind == "ExternalInput":
            if name != pname:
                in_names.append(name)
        elif alloc.kind == "ExternalOutput":
            out_avals.append(jax.core.ShapedArray(
                tuple(alloc.tensor_shape), mybir.dt.np(alloc.dtype)))
            zero_outs.append(np.zeros(tuple(alloc.tensor_shape),
                                      mybir.dt.np(alloc.dtype)))
            out_names.append(name)
    n_params = len(in_names)
    n_outs = len(out_avals)
    all_names = in_names + out_names
    if pname is not None:
        all_names = all_names + [pname]

    donate = tuple(range(n_params, n_params + n_outs))

    def _body(*args):
        operands = list(args)
        if pname is not None:
            operands.append(bass2jax.partition_id_tensor())
        outs = bass2jax._bass_exec_p.bind(
            *operands, out_avals=tuple(out_avals), in_names=tuple(all_names),
            out_names=tuple(out_names), lowering_input_output_aliases=(),
            sim_require_finite=True, sim_require_nnan=True, nc=nc)
        return tuple(outs)

    devices = jax.devices()[:NCORES]
    mesh = Mesh(np.asarray(devices), ("core",))
    sharded = jax.jit(
        shard_map(_body, mesh=mesh,
                  in_specs=(PartitionSpec("core"),) * (n_params + n_outs),
                  out_specs=(PartitionSpec("core"),) * n_outs,
                  check_rep=False),
        donate_argnums=donate, keep_unused=True)

    def run(in_maps):
        per_core = [[np.asarray(m[n]) for n in in_names] for m in in_maps]
        concat_in = [np.concatenate([per_core[c][i] for c in range(NCORES)], 0)
                     for i in range(n_params)]
        concat_zeros = [np.zeros((NCORES * z.shape[0], *z.shape[1:]), z.dtype)
                        for z in zero_outs]
        out_arrs = sharded(*concat_in, *concat_zeros)
        out_arrs = jax.block_until_ready(out_arrs)
        return [{n: np.asarray(out_arrs[i]).reshape(NCORES, *out_avals[i].shape)[c]
                 for i, n in enumerate(out_names)} for c in range(NCORES)]

    return run


def _in_maps(returns, volumes, fractal_features):
    feat = np.concatenate([returns, volumes, fractal_features], axis=-1)  # [B,S,306]
    in_maps = []
    for c in range(NCORES):
        fc = feat[c * BL:(c + 1) * BL].reshape(T, I_IN).T  # [306, T]
        fp = np.zeros((IP, T), np.float32)
        fp[:I_IN] = fc
        in_maps.append({"feat": fp.astype(BFNP)})
    return in_maps


def kernel(returns, volumes, fractal_features, params, _dbg=False, _bench=0):
    returns = np.asarray(returns, np.float32)
    volumes = np.asarray(volumes, np.float32)
    fractal_features = np.asarray(fractal_features, np.float32)
    nc = _get_nc(params, dbg=_dbg)
    key = ("runner", _dbg)
    if key not in _CACHE:
        _CACHE[key] = _make_runner(nc)
    run = _CACHE[key]
    in_maps = _in_maps(returns, volumes, fractal_features)
    outs = run(in_maps)
    if _bench:
        import time
        times = []
        for _ in range(_bench):
            t0 = time.perf_counter()
            outs = run(in_maps)
            times.append(time.perf_counter() - t0)
        kernel._bench_times = times
    ret = np.concatenate([o["out_ret"] for o in outs], 0)
    reg = np.concatenate([o["out_reg"] for o in outs], 0)
    hur = np.concatenate([o["out_hur"] for o in outs], 0)
    fin = np.concatenate([o["out_final"] for o in outs], 0)
    if _dbg:
        kernel._last_outs = outs
    return ret, reg, hur, fin
